# revision 12
# baseline (speedup 1.0000x reference)
"""GCN layer on 8 trn2 NeuronCores — dense one-hot routing matmuls (v5).

out = segment_sum((h @ W * norm)[src], dst) * norm + bias

Algebra: (h@W)*norm = (h*norm)@W and segment_sum is linear, so with
h' = h*norm:  out = (segment_sum(h'[src], dst) @ W) * norm[dst] + bias.

v3..v4 used gpsimd dma_gather for h'[src]; measured HW throughput is
~250-300 ns per 256B descriptor regardless of source (HBM random, HBM
const-index, SBUF-source) and of queue count / single_packet — the SWDGE
descriptor path is per-descriptor-bound, ~64-90 ms for 1.6M edges. v5
eliminates gathers entirely:

  aggT[f, dcol] = sum_s  h'_s^T @ onehot_s  (dense routing matmuls)

Nodes are sharded by dst across cores (12800 padded dst cols each). Each
core streams ALL src tiles (h' bf16, sequential DMA) and, per src tile s
and per 512-col dst window w, builds a {0,1} one-hot routing block
pt[p, j] = (j == dstcol[p, s, w]) on DVE/Pool (int16 iota compare), then
PE accumulates  bank_w += h'_s^T @ pt  in PSUM across all s. Per-window
epilogue: scale by norm[dst] (column broadcast), GEMM with W, bias via
ACT, DMA out (transposed [f, dst]; host untransposes).

A src row may have at most one edge per (row, window) — extra edges are
remapped on the host to duplicate rows (node copies appended after the
real tiles, ~6-8% extra), so one tensor_scalar per (s, w) suffices.
Routing blocks alternate DVE (2/3, 16-bit 2x rate) and Pool (1/3).
"""
import numpy as np
import ml_dtypes

import concourse.bass as bass
import concourse.mybir as mybir
import concourse.tile as tile
from concourse import bacc
from concourse.bass_utils import run_bass_kernel_spmd

P = 128
N = 100000
E = 1600000
NCORES = 8
DPC = 12544              # real dst cols per core (98 tiles)
W = 512                  # dst window = one PSUM bank
NW = 25                  # windows per core (12800 padded dst cols)
DPAD = NW * W            # padded dst cols per core
NT0 = 784                # src tiles before duplicates (784*128 = 100352)
NPAD0 = NT0 * P
CHUNK = 32               # src tiles per h DMA
PASS_W = 7               # windows per pass (7 PSUM banks + 1 for GEMM)

BF = mybir.dt.bfloat16
I16 = mybir.dt.int16

_cache = {}
RUN_KWARGS = {}
LAST_RESULTS = None
LAST_NC = None
LAST_IN_MAPS = None
EMULATE = False
POOL_FRAC = 3            # every POOL_FRAC-th routing block builds on Pool


def _passes():
    out = []
    w0 = 0
    while w0 < NW:
        out.append(list(range(w0, min(w0 + PASS_W, NW))))
        w0 += PASS_W
    return out


def _build_program(tmax):
    nchunks = (tmax + CHUNK - 1) // CHUNK
    assert nchunks * CHUNK == tmax
    f32 = mybir.dt.float32

    nc = bacc.Bacc(None, target_bir_lowering=False)
    h_d = nc.dram_tensor("hp", [nchunks, P, CHUNK * P], BF, kind="ExternalInput")
    dc_d = nc.dram_tensor("dcol", [P, tmax * NW], mybir.dt.float32,
                           kind="ExternalInput")
    io_d = nc.dram_tensor("iota", [P, W], I16, kind="ExternalInput")
    nb_d = nc.dram_tensor("nb", [P, DPAD], BF, kind="ExternalInput")
    w_d = nc.dram_tensor("wt", [P, P], BF, kind="ExternalInput")
    bb_d = nc.dram_tensor("bb", [P, 1], f32, kind="ExternalInput")
    out_d = nc.dram_tensor("out", [NW, P, W], f32, kind="ExternalOutput")

    with tile.TileContext(nc) as tc:
        with (
            tc.tile_pool(name="const", bufs=1) as cpool,
            tc.tile_pool(name="h", bufs=3) as hpool,
            tc.tile_pool(name="pt", bufs=12) as ptpool,
            tc.tile_pool(name="bank", bufs=PASS_W, space="PSUM") as bkpool,
            tc.tile_pool(name="ps2", bufs=1, space="PSUM") as ps2pool,
            tc.tile_pool(name="ag", bufs=2) as agpool,
            tc.tile_pool(name="oo", bufs=2) as opool,
        ):
            dc_sb = cpool.tile([P, tmax * NW], mybir.dt.float32)
            nc.sync.dma_start(dc_sb[:], dc_d[:])
            io_sb = cpool.tile([P, W], I16)
            nc.sync.dma_start(io_sb[:], io_d[:])
            nb_sb = cpool.tile([P, DPAD], BF)
            nc.sync.dma_start(nb_sb[:], nb_d[:])
            w_sb = cpool.tile([P, P], BF)
            nc.sync.dma_start(w_sb[:], w_d[:])
            bb_sb = cpool.tile([P, 1], f32)
            nc.sync.dma_start(bb_sb[:], bb_d[:])

            bid = 0  # routing-block counter for DVE/Pool alternation
            for windows in _passes():
                banks = [bkpool.tile([P, W], f32, tag="bk", name=f"bk{wi}")
                         for wi in range(len(windows))]
                for ci in range(nchunks):
                    ht = hpool.tile([P, CHUNK, P], BF, tag="h")
                    nc.sync.dma_start(ht[:], h_d[ci])
                    for t in range(CHUNK):
                        s = ci * CHUNK + t
                        for wi, w in enumerate(windows):
                            pt = ptpool.tile([P, W], BF, tag="pt")
                            eng = (nc.gpsimd if bid % POOL_FRAC == 0
                                   else nc.vector)
                            eng.tensor_scalar(
                                pt[:], io_sb[:],
                                dc_sb[:, s * NW + w:s * NW + w + 1], None,
                                op0=mybir.AluOpType.is_equal)
                            bid += 1
                            nc.tensor.matmul(
                                banks[wi][:], lhsT=ht[:, t, :], rhs=pt[:],
                                start=(s == 0), stop=(s == tmax - 1))
                for wi, w in enumerate(windows):
                    ag = agpool.tile([P, W], BF, tag="ag")
                    nc.vector.tensor_tensor(
                        ag[:], banks[wi][:], nb_sb[:, w * W:(w + 1) * W],
                        op=mybir.AluOpType.mult)
                    op = ps2pool.tile([P, W], f32, tag="o2")
                    nc.tensor.matmul(op[:], lhsT=w_sb[:], rhs=ag[:],
                                     start=True, stop=True)
                    o_sb = opool.tile([P, W], f32, tag="o")
                    nc.scalar.activation(
                        o_sb[:], op[:],
                        mybir.ActivationFunctionType.Identity,
                        bias=bb_sb[:, 0:1], scale=1.0)
                    nc.sync.dma_start(out_d[w], o_sb[:])
    nc.compile()
    return nc


def _prep_host(h, norm, src, dst, weight, bias):
    """Per-core inputs: dedup collisions via node duplication, build tables."""
    hp = (h * norm[:, None]).astype(np.float32)  # h' = h*norm, [N,128]

    per_core = []
    tmaxes = []
    for c in range(NCORES):
        d0 = c * DPC
        m = (dst >= d0) & (dst < d0 + DPC)
        es = src[m].astype(np.int64)
        dcol = (dst[m] - d0).astype(np.int64)
        w = dcol // W
        # rank of each edge within its (src row, window) group
        key = es * NW + w
        order = np.argsort(key, kind="stable")
        ks = key[order]
        first = np.ones(len(ks), bool)
        first[1:] = ks[1:] != ks[:-1]
        grp_start = np.maximum.accumulate(np.where(first, np.arange(len(ks)), 0))
        rank = np.arange(len(ks)) - grp_start
        rank_e = np.empty(len(ks), np.int64)
        rank_e[order] = rank
        # dup rows: node r needs max_w(count) - 1 copies; copy k-1 serves rank k
        need = rank_e > 0
        if need.any():
            # per (node, rank) pair -> one dup row
            pair = es[need] * 64 + np.minimum(rank_e[need], 63)
            upair, inv = np.unique(pair, return_inverse=True)
            dup_src = (upair // 64).astype(np.int64)
            newrow = NPAD0 + inv
            row = es.copy()
            row[need] = newrow
            ndup = len(upair)
        else:
            row = es
            dup_src = np.zeros(0, np.int64)
            ndup = 0
        per_core.append((row, w, dcol % W, dup_src))
        tmaxes.append((NPAD0 + ndup + P - 1) // P)

    tmax = max(tmaxes)
    tmax = ((tmax + CHUNK - 1) // CHUNK) * CHUNK
    npad = tmax * P

    in_maps = []
    for c in range(NCORES):
        row, w, wincol, dup_src = per_core[c]
        hpc = np.zeros((npad, P), np.float32)
        hpc[:N] = hp
        hpc[NPAD0:NPAD0 + len(dup_src)] = hp[dup_src]
        hb = hpc.astype(ml_dtypes.bfloat16)
        # chunk-interleaved layout: [nchunk, 128 p, CHUNK t, 128 f]
        hb = np.ascontiguousarray(
            hb.reshape(tmax // CHUNK, CHUNK, P, P).transpose(0, 2, 1, 3)
            .reshape(tmax // CHUNK, P, CHUNK * P))

        dc = np.full((P, tmax * NW), -1, np.float32)
        p_l = (row % P).astype(np.int64)
        s_l = (row // P).astype(np.int64)
        dc[p_l, s_l * NW + w] = wincol.astype(np.float32)

        nb = np.zeros((DPAD,), np.float32)
        d0 = c * DPC
        nvalid = min(DPC, max(0, N - d0))
        nb[:nvalid] = norm[d0:d0 + nvalid]
        nb_b = np.tile(nb.astype(ml_dtypes.bfloat16)[None, :], (P, 1))

        iota = np.tile(np.arange(W, dtype=np.int16)[None, :], (P, 1))

        in_maps.append({
            "hp": hb,
            "dcol": dc,
            "iota": iota,
            "nb": nb_b,
            "wt": weight.astype(ml_dtypes.bfloat16),
            "bb": bias.reshape(P, 1).astype(np.float32),
        })
    return tmax, in_maps


def _emulate_core(m, tmax):
    """Numpy mirror of the device program for one core."""
    nch = tmax // CHUNK
    hb = m["hp"].reshape(nch, P, CHUNK, P).transpose(0, 2, 1, 3) \
        .reshape(tmax, P, P).astype(np.float32)        # [s][p][f]
    dc = m["dcol"]                                     # [128, tmax*NW]
    nb = m["nb"][0].astype(np.float32)                 # [DPAD]
    wt = m["wt"].astype(np.float32)
    bb = m["bb"].reshape(-1)
    iota = np.arange(W)
    out = np.zeros((NW, P, W), np.float32)
    for w in range(NW):
        bank = np.zeros((P, W), np.float32)
        for s in range(tmax):
            col = dc[:, s * NW + w]                    # [128]
            pt = (iota[None, :] == col[:, None]).astype(np.float32)
            pt = pt.astype(ml_dtypes.bfloat16).astype(np.float32)
            bank += hb[s].T @ pt
        ag = (bank * nb[None, w * W:(w + 1) * W]).astype(ml_dtypes.bfloat16)
        out[w] = wt.T @ ag.astype(np.float32) + bb[:, None]
    return out


def kernel(h, norm, src, dst, weight, bias):
    h = np.ascontiguousarray(h, dtype=np.float32)
    norm = np.ascontiguousarray(norm, dtype=np.float32).reshape(-1)
    src = np.ascontiguousarray(src, dtype=np.int64).reshape(-1)
    dst = np.ascontiguousarray(dst, dtype=np.int64).reshape(-1)
    weight = np.ascontiguousarray(weight, dtype=np.float32)
    bias = np.ascontiguousarray(bias, dtype=np.float32).reshape(-1)
    assert h.shape == (N, P) and src.shape[0] == E

    tmax, in_maps = _prep_host(h, norm, src, dst, weight, bias)

    if EMULATE:
        results = [_emulate_core(m, tmax) for m in in_maps]
    else:
        if tmax not in _cache:
            _cache[tmax] = _build_program(tmax)
        nc = _cache[tmax]
        global LAST_NC, LAST_IN_MAPS, LAST_RESULTS
        LAST_NC, LAST_IN_MAPS = nc, in_maps
        res = run_bass_kernel_spmd(nc, in_maps, core_ids=list(range(NCORES)),
                                   **RUN_KWARGS)
        LAST_RESULTS = res
        results = [res.results[c]["out"] for c in range(NCORES)]

    out = np.zeros((N, P), np.float32)
    for c in range(NCORES):
        oc = results[c].reshape(NW * 1, P, W).transpose(0, 2, 1) \
            .reshape(DPAD, P)                          # [dcol, f]
        d0 = c * DPC
        nvalid = min(DPC, max(0, N - d0))
        out[d0:d0 + nvalid] = oc[:nvalid]
    return out
